# revision 1
# baseline (speedup 1.0000x reference)
"""Trainium2 Bass kernel for nn_CrossAttentionFusion.

The reference module is a cross-attention with seq_len==1 on both sides:
softmax over a single key is identically 1, so the Q/K projections are
dead code and the whole module collapses to

    y[b,0] = LN(x[b,0] + x[b,1] @ Weff.T + beff)
    y[b,1] = LN(x[b,1] + x[b,0] @ Weff.T + beff)

with Weff = Wo @ Wv, beff = Wo @ bv + bo.  This is a memory-bound
[1M x 256] x [256 x 256] matmul + residual + LayerNorm.

Distribution: pure data parallel over batch across 8 NeuronCores.

v4 design (vs v1: fp32, on-device PE transposes, per-plane bn stats):
  - bf16 on the wire.  rel-err budget is 2e-2; bf16 rounding contributes
    ~2e-3.  Halves HBM traffic: the per-core DMA floor drops from ~750us
    to ~375us.
  - x is pre-transposed host-side into [tiles, 512 feat, G*128 pairs]
    (feature-major, >=512B contiguous runs).  The PE consumes transposed
    chunks directly as the stationary operand, so v1's transpose matmuls
    and the PSUM->SBUF copy pass disappear.
  - The two output planes are INTERLEAVED column-wise in PSUM
    (col 2f+h = plane h, feature f) by interleaving the moving-operand
    columns host-side.  bn_stats natively produces separate stats for
    even and odd columns, so ONE single-group bn_stats per 128-pair tile
    yields both planes' mean/var with no aggregation pass at all.
  - Per tile: 4 bf16 matmuls (or 8 narrow ones with split_mm, saving PE
    cycles on the residual identity halves), one bn_stats, a tiny stats
    tail (rstd = (var+eps)^-0.5, nmr = -mean*rstd) on the otherwise-idle
    GPSIMD engine, and two ACT Identity activations apply y*rstd + nmr
    per plane while de-interleaving PSUM fp32 -> SBUF bf16.
  - Stores are issued store_delay megatiles late so their semaphore wait
    never blocks the SP sequencer from issuing loads ahead.
  - Host converts the bf16 output back to fp32.
"""

import sys

import numpy as np

sys.path.insert(0, "/opt/trn_rl_repo")

import concourse.bass as bass  # noqa: E402
import concourse.tile as tile  # noqa: E402
from concourse import bass_utils, mybir  # noqa: E402

B = 524288
D = 256
N_CORES = 8
LN_EPS = 1e-5

_F32 = mybir.dt.float32
_BF16 = mybir.dt.bfloat16


def split_waits(nc, limit=1):
    """Hoist excess sync waits onto single-wait EventSemaphore instructions.

    The walrus in this toolchain rejects instructions carrying more than one
    sync wait ("Too many sync wait commands"), so we post-process the BIR:
    for any instruction with >limit waits, emit preceding EventSemaphore
    instructions (same engine, program order) each carrying one wait.
    """
    n_new = 0
    for f in nc.m.functions:
        for bb in f.blocks:
            out = []
            for inst in bb.instructions:
                si = getattr(inst, "sync_info", None)
                ow = list(si.on_wait) if (si is not None and si.on_wait) else []
                if len(ow) > limit:
                    for k, w in enumerate(ow[:-limit]):
                        es = mybir.InstEventSemaphore(
                            name=f"{inst.name}_ws{k}",
                            engine=inst.engine,
                            ins=[], outs=[],
                            sync_info=mybir.SyncInfo(on_wait=[w], on_update=[]),
                        )
                        nc.inst_map[es.name] = es
                        out.append(es)
                        n_new += 1
                    inst.sync_info = mybir.SyncInfo(
                        on_wait=ow[-limit:], on_update=list(si.on_update or []))
                out.append(inst)
            bb.instructions[:] = out
    return n_new


def build_nc(n, G=2, with_bias=False, with_gamma=False, with_beta=False,
             repeats=1, bufs=None, skip=(), store_eng="sync",
             norm_split=3, split_mm=False, tail="actdve", store_delay=2,
             sg=2):
    """Build the per-core Bass program for n batch pairs (2*n rows).

    Inputs (DRAM, bf16):
      xt    [n//(128*G), 2*D, G*128]  pre-transposed x megatiles
            (feature-major: f = plane*D + feat, columns = pair index)
      wt/ident (split_mm) or wcat     moving operands (plane-interleaved)
      out   [n, 2*D]                  y, natural row-major (bf16)

    Options:
      norm_split=k  every k-th tile runs the plane-0 normalize on DVE
                    (fine-grained ACT/DVE balance); 0/False = all on ACT
      tail          'pool' (gpsimd pow), 'actdve' (ACT sqrt + DVE recip)
      store_delay   stores issue this many megatiles late (SP never
                    blocks on them)
    repeats>1 wraps the body in For_i for repeat-amplified wall-clock
    timing (idempotent work).
    """
    assert n % (128 * G) == 0, (n, G)
    n_mega = n // (128 * G)
    bf = {"xt": 4, "yout": 6, "st": 4, "yps": 8}
    if bufs:
        bf.update(bufs)
    SG = min(sg, G)  # stats sub-group: tiles sharing one rstd/nmr batch

    nc = bass.Bass(trn_type="TRN2")
    xt_d = nc.dram_tensor("xt", [n_mega, 2 * D, G * 128], _BF16,
                          kind="ExternalInput")
    if split_mm == "hybrid":
        wt_d = nc.dram_tensor("wt", [2, 128, D], _BF16, kind="ExternalInput")
        id_d = nc.dram_tensor("ident", [128, 128], _BF16,
                              kind="ExternalInput")
        wc0_d = nc.dram_tensor("wcat0", [128, 2 * D], _BF16,
                               kind="ExternalInput")
    elif split_mm:
        wt_d = nc.dram_tensor("wt", [2, 128, D], _BF16, kind="ExternalInput")
        id_d = nc.dram_tensor("ident", [128, 128], _BF16,
                              kind="ExternalInput")
    else:
        wcat_d = nc.dram_tensor("wcat", [2, 2, 128, 2 * D], _BF16,
                                kind="ExternalInput")
    if with_bias:
        beff_d = nc.dram_tensor("beff", [1, 2 * D], _BF16,
                                kind="ExternalInput")
    if with_gamma:
        gamma_d = nc.dram_tensor("gamma", [1, D], _F32, kind="ExternalInput")
    if with_beta:
        beta_d = nc.dram_tensor("beta", [1, D], _F32, kind="ExternalInput")
    out_d = nc.dram_tensor("out", [n, 2 * D], _BF16, kind="ExternalOutput")

    with tile.TileContext(nc) as tc:
        with (
            tc.tile_pool(name="const", bufs=1) as constp,
            tc.tile_pool(name="xt", bufs=bf["xt"]) as xtp,
            tc.tile_pool(name="yout", bufs=bf["yout"]) as youtp,
            tc.tile_pool(name="st", bufs=bf["st"]) as statp,
            tc.tile_pool(name="yps", bufs=bf["yps"], space="PSUM") as ypsum,
        ):
            if split_mm:
                wt_sb = constp.tile([128, 2, D], _BF16)
                nc.sync.dma_start(
                    out=wt_sb, in_=wt_d[:].rearrange("c p f -> p c f"))
                id_sb = constp.tile([128, 128], _BF16)
                nc.sync.dma_start(out=id_sb, in_=id_d[:])
                if split_mm == "hybrid":
                    wc0_sb = constp.tile([128, 2 * D], _BF16)
                    nc.sync.dma_start(out=wc0_sb, in_=wc0_d[:])
            else:
                wcat_sb = constp.tile([128, 2, 2, 2 * D], _BF16)
                nc.sync.dma_start(
                    out=wcat_sb, in_=wcat_d[:].rearrange("p c q f -> q p c f"))
            if tail != "pool":
                eps_sb = constp.tile([128, 1], _F32)
                nc.vector.memset(eps_sb, LN_EPS)
            if with_bias:
                ones_sb = constp.tile([1, 128], _BF16)
                nc.vector.memset(ones_sb, 1.0)
                beff_sb = constp.tile([1, 2 * D], _BF16)
                nc.sync.dma_start(out=beff_sb, in_=beff_d[:])
            if with_gamma:
                gamma_sb = constp.tile([128, D], _F32)
                nc.gpsimd.dma_start(
                    out=gamma_sb,
                    in_=bass.AP(tensor=gamma_d[:].tensor, offset=0,
                                ap=[[0, 128], [1, D]]),
                )
            if with_beta:
                beta_sb = constp.tile([128, D], _F32)
                nc.gpsimd.dma_start(
                    out=beta_sb,
                    in_=bass.AP(tensor=beta_d[:].tensor, offset=0,
                                ap=[[0, 128], [1, D]]),
                )

            mul = mybir.AluOpType.mult

            def megatile(m):
                r0 = m * G * 128
                # ---- load pre-transposed megatile ----
                xt = xtp.tile([128, 4, G * 128], _BF16)
                nc.sync.dma_start(
                    out=xt,
                    in_=xt_d[m].rearrange("(c p) b -> p c b", p=128),
                )
                yo = youtp.tile([128, G, 2 * D], _BF16)
                for g0 in range(0, G, SG):
                    megagroup(m, g0, xt, yo)
                pending.append((r0, yo))
                if len(pending) > store_delay:
                    emit_store(*pending.pop(0))

            def emit_store(r0, yo):
                eng = {"sync": nc.sync, "scalar": nc.scalar,
                       "gpsimd": nc.gpsimd, "vector": nc.vector}[store_eng]
                eng.dma_start(
                    out=out_d[r0:r0 + G * 128, :].rearrange(
                        "(g p) f -> p g f", p=128),
                    in_=yo,
                )

            def megagroup(m, g0, xt, yo):
                st = statp.tile([128, SG, 6], _F32)
                rstd = statp.tile([128, SG, 2], _F32)
                nmr = statp.tile([128, SG, 2], _F32)
                yps = []
                for sj in range(SG):
                    j = g0 + sj
                    # ---- y = x + xswap @ Weff.T, planes col-interleaved ----
                    yp = ypsum.tile([128, 2 * D], _F32)
                    ypv = yp.rearrange("p (f h) -> p h f", h=2)
                    yps.append((yp, ypv))
                    bs = slice(j * 128, (j + 1) * 128)
                    if "mm" in skip:
                        nc.tensor.matmul(
                            out=(ypv[:, 0, :] if split_mm else yp),
                            lhsT=xt[:, 0, bs],
                            rhs=(wt_sb[:, 0, :] if split_mm
                                 else wcat_sb[:, 0, 0, :]),
                            start=True, stop=True, skip_group_check=True)
                    elif split_mm == "hybrid":
                        # mm1 full-width (start=True, initializes every
                        # slot: x0c0 residual into even cols + x0c0@Weff
                        # into odd cols); then narrow matmuls purely
                        # accumulate, saving PE stream cycles on the
                        # identity halves.
                        nc.tensor.matmul(
                            out=yp, lhsT=xt[:, 0, bs], rhs=wc0_sb,
                            start=True, stop=False, skip_group_check=True)
                        for ch, idp, idc, wtp in [(1, 0, 1, 1),
                                                  (2, 1, 0, 0),
                                                  (3, 1, 1, 0)]:
                            nc.tensor.matmul(
                                out=ypv[:, idp, idc * 128:(idc + 1) * 128],
                                lhsT=xt[:, ch, bs], rhs=id_sb,
                                start=False, stop=False,
                                skip_group_check=True)
                            nc.tensor.matmul(
                                out=ypv[:, wtp, :],
                                lhsT=xt[:, ch, bs],
                                rhs=wt_sb[:, idc, :],
                                start=False,
                                stop=(ch == 3 and not with_bias),
                                skip_group_check=True)
                    elif split_mm:
                        # residual: x_p chunk c -> plane p cols c-blk
                        # (interleaved).  Only the first matmul has
                        # start=True: it marks the whole 2KiB PSUM
                        # zero-region pending, later sub-region writes
                        # overwrite-on-first-touch, Weff matmuls accumulate.
                        for p in range(2):
                            for c in range(2):
                                nc.tensor.matmul(
                                    out=ypv[:, p, c * 128:(c + 1) * 128],
                                    lhsT=xt[:, 2 * p + c, bs],
                                    rhs=id_sb,
                                    start=(p == 0 and c == 0), stop=False,
                                    skip_group_check=True,
                                )
                        # fused: x_p chunks @ Weff.T -> other plane
                        for p, c in [(1, 0), (1, 1), (0, 0), (0, 1)]:
                            nc.tensor.matmul(
                                out=ypv[:, 1 - p, :],
                                lhsT=xt[:, 2 * p + c, bs],
                                rhs=wt_sb[:, c, :],
                                start=False,
                                stop=(p == 0 and c == 1 and not with_bias),
                                skip_group_check=True,
                            )
                    else:
                        for i, (p, c) in enumerate([(0, 0), (0, 1),
                                                    (1, 0), (1, 1)]):
                            nc.tensor.matmul(
                                out=yp,
                                lhsT=xt[:, 2 * p + c, bs],
                                rhs=wcat_sb[:, p, c, :],
                                start=(i == 0),
                                stop=(i == 3 and not with_bias),
                                skip_group_check=True,
                            )
                    if with_bias:
                        nc.tensor.matmul(
                            out=yp,
                            lhsT=ones_sb,
                            rhs=beff_sb,
                            start=False, stop=True, skip_group_check=True,
                        )
                    # ---- LayerNorm stats: even cols = plane0, odd =
                    # plane1, so one single-group bn_stats gives both ----
                    if "stats" not in skip:
                        nc.vector.bn_stats(out=st[:, sj], in_=yp)

                # ---- stats tail: rstd = (cv/D+eps)^-0.5, nmr = -m*rstd ----
                # st[:, :, 1::3] = means, st[:, :, 2::3] = count*var
                if "stats" not in skip:
                    if tail == "pool":
                        nc.gpsimd.tensor_scalar(
                            out=rstd, in0=st[:, :, 2::3], scalar1=1.0 / D,
                            scalar2=LN_EPS, op0=mul, op1=mybir.AluOpType.add)
                        nc.gpsimd.tensor_scalar(
                            out=rstd, in0=rstd, scalar1=-0.5, scalar2=None,
                            op0=mybir.AluOpType.pow)
                        nc.gpsimd.scalar_tensor_tensor(
                            out=nmr, in0=st[:, :, 1::3], scalar=-1.0,
                            in1=rstd, op0=mul, op1=mul)
                    else:
                        nc.scalar.activation(
                            out=rstd, in_=st[:, :, 2::3],
                            func=mybir.ActivationFunctionType.Sqrt,
                            bias=eps_sb, scale=1.0 / D,
                        )
                        nc.vector.reciprocal(out=rstd, in_=rstd)
                        nc.vector.scalar_tensor_tensor(
                            out=nmr, in0=st[:, :, 1::3], scalar=-1.0,
                            in1=rstd, op0=mul, op1=mul)

                # ---- normalize: (y*rstd + nmr), de-interleave PSUM fp32
                # -> SBUF bf16 ----
                for sj in range(SG):
                    j = g0 + sj
                    yp, ypv = yps[sj]
                    for h in range(2):
                        if "stats" in skip or "norm" in skip:
                            nc.scalar.copy(out=yo[:, j, h * D:(h + 1) * D],
                                           in_=ypv[:, h, :])
                        elif (norm_split and h == 0
                              and (m * G + j) % int(norm_split) == 0):
                            nc.vector.tensor_scalar(
                                out=yo[:, j, h * D:(h + 1) * D],
                                in0=ypv[:, h, :],
                                scalar1=rstd[:, sj, h:h + 1],
                                scalar2=nmr[:, sj, h:h + 1],
                                op0=mul,
                                op1=mybir.AluOpType.add,
                            )
                        else:
                            nc.scalar.activation(
                                out=yo[:, j, h * D:(h + 1) * D],
                                in_=ypv[:, h, :],
                                func=mybir.ActivationFunctionType.Identity,
                                bias=nmr[:, sj, h:h + 1],
                                scale=rstd[:, sj, h:h + 1],
                            )
                    if with_gamma:
                        for h in range(2):
                            nc.vector.tensor_mul(
                                out=yo[:, j, h * D:(h + 1) * D],
                                in0=yo[:, j, h * D:(h + 1) * D],
                                in1=gamma_sb,
                            )
                    if with_beta:
                        for h in range(2):
                            nc.vector.tensor_add(
                                out=yo[:, j, h * D:(h + 1) * D],
                                in0=yo[:, j, h * D:(h + 1) * D],
                                in1=beta_sb,
                            )

            if repeats > 1:
                with tc.For_i(0, repeats, 1):
                    pending = []
                    for m in range(n_mega):
                        megatile(m)
                    for args in pending:
                        emit_store(*args)
            else:
                pending = []
                for m in range(n_mega):
                    megatile(m)
                for args in pending:
                    emit_store(*args)
    split_waits(nc)
    return nc


def _to_bf16(a):
    import ml_dtypes
    return a.astype(ml_dtypes.bfloat16)


def _prepare(inputs, G=2, split_mm=True):
    """Host-side prep: collapse weights, bf16 conversion, pre-transpose,
    interleave weight columns, shard across cores."""
    x = np.asarray(inputs["x"], dtype=np.float32)
    ipw = np.asarray(inputs["in_proj_w"], dtype=np.float32)
    ipb = np.asarray(inputs["in_proj_b"], dtype=np.float32)
    opw = np.asarray(inputs["out_proj_w"], dtype=np.float32)
    opb = np.asarray(inputs["out_proj_b"], dtype=np.float32)
    gamma = np.asarray(inputs["ln_gamma"], dtype=np.float32)
    beta = np.asarray(inputs["ln_beta"], dtype=np.float32)

    d = x.shape[2]
    wv = ipw[2 * d:3 * d]
    bv = ipb[2 * d:3 * d]
    weff_t = np.ascontiguousarray((opw @ wv).T)          # [in_f, out_f]
    beff = opw @ bv + opb                                # [out_f]

    with_bias = bool(np.any(beff != 0.0))
    with_gamma = bool(np.any(gamma != 1.0))
    with_beta = bool(np.any(beta != 0.0))

    nb = x.shape[0]
    per_core = nb // N_CORES
    tile_rows = 128 * G

    # Pre-transposed bf16 x: [n_tiles, 2*d (f-major: plane*d+feat), rows]
    xt = _to_bf16(x).reshape(nb // tile_rows, tile_rows, 2 * d)
    xt = np.ascontiguousarray(xt.swapaxes(1, 2))

    if split_mm:
        base = {
            "wt": _to_bf16(weff_t.reshape(2, 128, d)),
            "ident": _to_bf16(np.eye(128, dtype=np.float32)),
        }
        if split_mm == "hybrid":
            eye = np.eye(d, dtype=np.float32)
            wc0 = np.empty((128, 2 * d), dtype=np.float32)
            wc0[:, 0::2] = eye[0:128]
            wc0[:, 1::2] = weff_t[0:128]
            base["wcat0"] = _to_bf16(wc0)
    else:
        # moving operands with plane-interleaved columns: for lhsT = x_p
        # chunk c, out col 2f+h gets (identity if h==p else Weff.T) col f
        eye = np.eye(d, dtype=np.float32)
        wcat = np.empty((2, 2, 128, 2 * d), dtype=np.float32)
        for c in range(2):
            rows = slice(c * 128, (c + 1) * 128)
            wcat[0, c, :, 0::2] = eye[rows]
            wcat[0, c, :, 1::2] = weff_t[rows]
            wcat[1, c, :, 0::2] = weff_t[rows]
            wcat[1, c, :, 1::2] = eye[rows]
        base = {"wcat": _to_bf16(wcat)}
    if with_bias:
        base["beff"] = _to_bf16(np.repeat(beff, 2).reshape(1, 2 * d))
    if with_gamma:
        base["gamma"] = np.ascontiguousarray(gamma.reshape(1, d))
    if with_beta:
        base["beta"] = np.ascontiguousarray(beta.reshape(1, d))

    tiles_per_core = per_core // tile_rows
    in_maps = []
    for c in range(N_CORES):
        m = dict(base)
        m["xt"] = xt[c * tiles_per_core:(c + 1) * tiles_per_core]
        in_maps.append(m)
    return in_maps, per_core, (with_bias, with_gamma, with_beta), x.shape


def kernel(x, in_proj_w, in_proj_b, out_proj_w, out_proj_b, ln_gamma, ln_beta,
           _trace=False, _G=2, _opts=None):
    opts = dict(split_mm=False)
    if _opts:
        opts.update(_opts)
    inputs = dict(x=x, in_proj_w=in_proj_w, in_proj_b=in_proj_b,
                  out_proj_w=out_proj_w, out_proj_b=out_proj_b,
                  ln_gamma=ln_gamma, ln_beta=ln_beta)
    in_maps, per_core, (wb, wg, wbt), xshape = _prepare(
        inputs, G=_G, split_mm=opts["split_mm"])
    nc = build_nc(per_core, G=_G, with_bias=wb, with_gamma=wg, with_beta=wbt,
                  **opts)
    res = bass_utils.run_bass_kernel_spmd(
        nc, in_maps, core_ids=list(range(N_CORES)), trace=_trace,
    )
    out = np.concatenate([r["out"] for r in res.results], axis=0)
    kernel.last_results = res
    return out.astype(np.float32).reshape(xshape)



# revision 21
# speedup vs baseline: 8.0492x; 8.0492x over previous
"""Trainium2 Bass kernel for nn_CrossAttentionFusion.

The reference module is a cross-attention with seq_len==1 on both sides:
softmax over a single key is identically 1, so the Q/K projections are
dead code and the whole module collapses to

    y[b,0] = LN(x[b,0] + x[b,1] @ Weff.T + beff)
    y[b,1] = LN(x[b,1] + x[b,0] @ Weff.T + beff)

with Weff = Wo @ Wv, beff = Wo @ bv + bo.  This is a memory-bound
[1M x 256] x [256 x 256] matmul + residual + LayerNorm.

Distribution: pure data parallel over batch across 8 NeuronCores.

v4 design (vs v1: fp32, on-device PE transposes, per-plane bn stats):
  - bf16 on the wire.  rel-err budget is 2e-2; bf16 rounding contributes
    ~2e-3.  Halves HBM traffic: the per-core DMA floor drops from ~750us
    to ~375us.
  - x is pre-transposed host-side into [tiles, 512 feat, G*128 pairs]
    (feature-major, >=512B contiguous runs).  The PE consumes transposed
    chunks directly as the stationary operand, so v1's transpose matmuls
    and the PSUM->SBUF copy pass disappear.
  - The two output planes are INTERLEAVED column-wise in PSUM
    (col 2f+h = plane h, feature f) by interleaving the moving-operand
    columns host-side.  bn_stats natively produces separate stats for
    even and odd columns, so ONE single-group bn_stats per 128-pair tile
    yields both planes' mean/var with no aggregation pass at all.
  - Per tile: 4 bf16 matmuls (or 8 narrow ones with split_mm, saving PE
    cycles on the residual identity halves), one bn_stats, a tiny stats
    tail (rstd = (var+eps)^-0.5, nmr = -mean*rstd) on the otherwise-idle
    GPSIMD engine, and two ACT Identity activations apply y*rstd + nmr
    per plane while de-interleaving PSUM fp32 -> SBUF bf16.
  - Stores are issued store_delay megatiles late so their semaphore wait
    never blocks the SP sequencer from issuing loads ahead.
  - Host converts the bf16 output back to fp32.

v5 exploration notes (all measured with interleaved on-device repeat-slope
timing; axon dispatch jitter is bimodal ~45-95ms so single-run slopes lie):
  - True body time of this default config is ~560-580us/core.  It is a
    multi-engine equilibrium: skip'ing 3 of the 4 matmuls saves only ~5%,
    G=4/G=8 DMA chunking and partition-contiguous DRAM layouts (xt_pc/
    out_pc) are neutral, larger or smaller pool bufs are worse.
  - split_mm='paired' (8 narrow MMs, chunk-major so each stationary loads
    once) is ~2x worse despite 25% fewer streamed columns: LDWEIGHTS and
    PSUM sub-region bookkeeping dominate (walrus runs --enable-ldw-opt=false).
  - Offloading normalize to GPSIMD (norm_pat='C...' + ACT interleaved copy)
    is ~4x worse: generic Pool elementwise costs ~2us/instruction.  Pool
    also rejects scalar_tensor_tensor and pow at walrus codegen.
  - bn_stats is DVE-only, BN_STATS_FMAX=512 forbids batching two tiles.
  - norm_split in {2,3,5} and tail placement are within noise (~±1%).
"""

import sys

import numpy as np

sys.path.insert(0, "/opt/trn_rl_repo")

import concourse.bass as bass  # noqa: E402
import concourse.tile as tile  # noqa: E402
from concourse import bass_utils, mybir  # noqa: E402

B = 524288
D = 256
N_CORES = 8
LN_EPS = 1e-5

_F32 = mybir.dt.float32
_BF16 = mybir.dt.bfloat16


def split_waits(nc, limit=1):
    """Hoist excess sync waits onto single-wait EventSemaphore instructions.

    The walrus in this toolchain rejects instructions carrying more than one
    sync wait ("Too many sync wait commands"), so we post-process the BIR:
    for any instruction with >limit waits, emit preceding EventSemaphore
    instructions (same engine, program order) each carrying one wait.
    """
    n_new = 0
    for f in nc.m.functions:
        for bb in f.blocks:
            out = []
            for inst in bb.instructions:
                si = getattr(inst, "sync_info", None)
                ow = list(si.on_wait) if (si is not None and si.on_wait) else []
                if len(ow) > limit:
                    for k, w in enumerate(ow[:-limit]):
                        es = mybir.InstEventSemaphore(
                            name=f"{inst.name}_ws{k}",
                            engine=inst.engine,
                            ins=[], outs=[],
                            sync_info=mybir.SyncInfo(on_wait=[w], on_update=[]),
                        )
                        nc.inst_map[es.name] = es
                        out.append(es)
                        n_new += 1
                    inst.sync_info = mybir.SyncInfo(
                        on_wait=ow[-limit:], on_update=list(si.on_update or []))
                out.append(inst)
            bb.instructions[:] = out
    return n_new


def build_nc(n, G=2, with_bias=False, with_gamma=False, with_beta=False,
             repeats=1, bufs=None, skip=(), store_eng="sync",
             norm_split=3, split_mm=False, tail="actdve", store_delay=2,
             sg=2, norm_pat=None, bn_src="psum", xt_pc=False, out_pc=False):
    """Build the per-core Bass program for n batch pairs (2*n rows).

    Inputs (DRAM, bf16):
      xt    [n//(128*G), 2*D, G*128]  pre-transposed x megatiles
            (feature-major: f = plane*D + feat, columns = pair index)
      wt/ident (split_mm) or wcat     moving operands (plane-interleaved)
      out   [n, 2*D]                  y, natural row-major (bf16)

    Options:
      norm_split=k  every k-th tile runs the plane-0 normalize on DVE
                    (fine-grained ACT/DVE balance); 0/False = all on ACT
      tail          'pool' (gpsimd pow), 'actdve' (ACT sqrt + DVE recip)
      store_delay   stores issue this many megatiles late (SP never
                    blocks on them)
    repeats>1 wraps the body in For_i for repeat-amplified wall-clock
    timing (idempotent work).
    """
    assert n % (128 * G) == 0, (n, G)
    n_mega = n // (128 * G)
    bf = {"xt": 4, "yout": 6, "st": 4, "yps": 8, "yi": 6}
    if bufs:
        bf.update(bufs)
    SG = min(sg, G)  # stats sub-group: tiles sharing one rstd/nmr batch

    nc = bass.Bass(trn_type="TRN2")
    if xt_pc:
        # partition-contiguous: each partition's megatile line is one run
        xt_d = nc.dram_tensor("xt", [n_mega, 128, 4, G * 128], _BF16,
                              kind="ExternalInput")
    else:
        xt_d = nc.dram_tensor("xt", [n_mega, 2 * D, G * 128], _BF16,
                              kind="ExternalInput")
    if split_mm == "hybrid":
        wt_d = nc.dram_tensor("wt", [2, 128, D], _BF16, kind="ExternalInput")
        id_d = nc.dram_tensor("ident", [128, 128], _BF16,
                              kind="ExternalInput")
        wc0_d = nc.dram_tensor("wcat0", [128, 2 * D], _BF16,
                               kind="ExternalInput")
    elif split_mm:
        wt_d = nc.dram_tensor("wt", [2, 128, D], _BF16, kind="ExternalInput")
        id_d = nc.dram_tensor("ident", [128, 128], _BF16,
                              kind="ExternalInput")
    else:
        wcat_d = nc.dram_tensor("wcat", [2, 2, 128, 2 * D], _BF16,
                                kind="ExternalInput")
    if with_bias:
        beff_d = nc.dram_tensor("beff", [1, 2 * D], _BF16,
                                kind="ExternalInput")
    if with_gamma:
        gamma_d = nc.dram_tensor("gamma", [1, D], _F32, kind="ExternalInput")
    if with_beta:
        beta_d = nc.dram_tensor("beta", [1, D], _F32, kind="ExternalInput")
    if out_pc:
        out_d = nc.dram_tensor("out", [n_mega, 128, G, 2 * D], _BF16,
                               kind="ExternalOutput")
    else:
        out_d = nc.dram_tensor("out", [n, 2 * D], _BF16,
                               kind="ExternalOutput")

    with tile.TileContext(nc) as tc:
        with (
            tc.tile_pool(name="const", bufs=1) as constp,
            tc.tile_pool(name="xt", bufs=bf["xt"]) as xtp,
            tc.tile_pool(name="yout", bufs=bf["yout"]) as youtp,
            tc.tile_pool(name="st", bufs=bf["st"]) as statp,
            tc.tile_pool(name="yps", bufs=bf["yps"], space="PSUM") as ypsum,
            tc.tile_pool(name="yi", bufs=bf["yi"]) as yip,
        ):
            if split_mm:
                wt_sb = constp.tile([128, 2, D], _BF16)
                nc.sync.dma_start(
                    out=wt_sb, in_=wt_d[:].rearrange("c p f -> p c f"))
                id_sb = constp.tile([128, 128], _BF16)
                nc.sync.dma_start(out=id_sb, in_=id_d[:])
                if split_mm == "hybrid":
                    wc0_sb = constp.tile([128, 2 * D], _BF16)
                    nc.sync.dma_start(out=wc0_sb, in_=wc0_d[:])
            else:
                wcat_sb = constp.tile([128, 2, 2, 2 * D], _BF16)
                nc.sync.dma_start(
                    out=wcat_sb, in_=wcat_d[:].rearrange("p c q f -> q p c f"))
            if tail != "pool":
                eps_sb = constp.tile([128, 1], _F32)
                nc.vector.memset(eps_sb, LN_EPS)
            if with_bias:
                ones_sb = constp.tile([1, 128], _BF16)
                nc.vector.memset(ones_sb, 1.0)
                beff_sb = constp.tile([1, 2 * D], _BF16)
                nc.sync.dma_start(out=beff_sb, in_=beff_d[:])
            if with_gamma:
                gamma_sb = constp.tile([128, D], _F32)
                nc.gpsimd.dma_start(
                    out=gamma_sb,
                    in_=bass.AP(tensor=gamma_d[:].tensor, offset=0,
                                ap=[[0, 128], [1, D]]),
                )
            if with_beta:
                beta_sb = constp.tile([128, D], _F32)
                nc.gpsimd.dma_start(
                    out=beta_sb,
                    in_=bass.AP(tensor=beta_d[:].tensor, offset=0,
                                ap=[[0, 128], [1, D]]),
                )

            mul = mybir.AluOpType.mult

            def megatile(m):
                r0 = m * G * 128
                # ---- load pre-transposed megatile ----
                xt = xtp.tile([128, 4, G * 128], _BF16)
                nc.sync.dma_start(
                    out=xt,
                    in_=(xt_d[m] if xt_pc
                         else xt_d[m].rearrange("(c p) b -> p c b", p=128)),
                )
                yo = youtp.tile([128, G, 2 * D], _BF16)
                for g0 in range(0, G, SG):
                    megagroup(m, g0, xt, yo)
                pending.append((r0, yo))
                if len(pending) > store_delay:
                    emit_store(*pending.pop(0))

            def emit_store(r0, yo):
                eng = {"sync": nc.sync, "scalar": nc.scalar,
                       "gpsimd": nc.gpsimd, "vector": nc.vector}[store_eng]
                eng.dma_start(
                    out=(out_d[r0 // (G * 128)] if out_pc
                         else out_d[r0:r0 + G * 128, :].rearrange(
                             "(g p) f -> p g f", p=128)),
                    in_=yo,
                )

            def megagroup(m, g0, xt, yo):
                st = statp.tile([128, SG, 6], _F32)
                rstd = statp.tile([128, SG, 2], _F32)
                nmr = statp.tile([128, SG, 2], _F32)
                yps = []
                yis = {}
                modes = {}
                for sj in range(SG):
                    j = g0 + sj
                    if norm_pat:
                        modes[sj] = norm_pat[(m * G + j) % len(norm_pat)]
                    else:
                        modes[sj] = None
                    # ---- y = x + xswap @ Weff.T, planes col-interleaved ----
                    yp = ypsum.tile([128, 2 * D], _F32)
                    ypv = yp.rearrange("p (f h) -> p h f", h=2)
                    yps.append((yp, ypv))
                    bs = slice(j * 128, (j + 1) * 128)
                    if "mm" in skip:
                        nc.tensor.matmul(
                            out=(ypv[:, 0, :] if split_mm else yp),
                            lhsT=xt[:, 0, bs],
                            rhs=(wt_sb[:, 0, :] if split_mm
                                 else wcat_sb[:, 0, 0, :]),
                            start=True, stop=True, skip_group_check=True)
                    elif split_mm == "hybrid":
                        # mm1 full-width (start=True, initializes every
                        # slot: x0c0 residual into even cols + x0c0@Weff
                        # into odd cols); then narrow matmuls purely
                        # accumulate, saving PE stream cycles on the
                        # identity halves.
                        nc.tensor.matmul(
                            out=yp, lhsT=xt[:, 0, bs], rhs=wc0_sb,
                            start=True, stop=False, skip_group_check=True)
                        for ch, idp, idc, wtp in [(1, 0, 1, 1),
                                                  (2, 1, 0, 0),
                                                  (3, 1, 1, 0)]:
                            nc.tensor.matmul(
                                out=ypv[:, idp, idc * 128:(idc + 1) * 128],
                                lhsT=xt[:, ch, bs], rhs=id_sb,
                                start=False, stop=False,
                                skip_group_check=True)
                            nc.tensor.matmul(
                                out=ypv[:, wtp, :],
                                lhsT=xt[:, ch, bs],
                                rhs=wt_sb[:, idc, :],
                                start=False,
                                stop=(ch == 3 and not with_bias),
                                skip_group_check=True)
                    elif split_mm == "paired":
                        # chunk-major: both matmuls of a chunk adjacent so
                        # the stationary operand is loaded once per chunk
                        for i, (p, c) in enumerate([(0, 0), (0, 1),
                                                    (1, 0), (1, 1)]):
                            nc.tensor.matmul(
                                out=ypv[:, p, c * 128:(c + 1) * 128],
                                lhsT=xt[:, 2 * p + c, bs],
                                rhs=id_sb,
                                start=(i == 0), stop=False,
                                skip_group_check=True,
                            )
                            nc.tensor.matmul(
                                out=ypv[:, 1 - p, :],
                                lhsT=xt[:, 2 * p + c, bs],
                                rhs=wt_sb[:, c, :],
                                start=False,
                                stop=(i == 3 and not with_bias),
                                skip_group_check=True,
                            )
                    elif split_mm:
                        # residual: x_p chunk c -> plane p cols c-blk
                        # (interleaved).  Only the first matmul has
                        # start=True: it marks the whole 2KiB PSUM
                        # zero-region pending, later sub-region writes
                        # overwrite-on-first-touch, Weff matmuls accumulate.
                        for p in range(2):
                            for c in range(2):
                                nc.tensor.matmul(
                                    out=ypv[:, p, c * 128:(c + 1) * 128],
                                    lhsT=xt[:, 2 * p + c, bs],
                                    rhs=id_sb,
                                    start=(p == 0 and c == 0), stop=False,
                                    skip_group_check=True,
                                )
                        # fused: x_p chunks @ Weff.T -> other plane
                        for p, c in [(1, 0), (1, 1), (0, 0), (0, 1)]:
                            nc.tensor.matmul(
                                out=ypv[:, 1 - p, :],
                                lhsT=xt[:, 2 * p + c, bs],
                                rhs=wt_sb[:, c, :],
                                start=False,
                                stop=(p == 0 and c == 1 and not with_bias),
                                skip_group_check=True,
                            )
                    else:
                        for i, (p, c) in enumerate([(0, 0), (0, 1),
                                                    (1, 0), (1, 1)]):
                            nc.tensor.matmul(
                                out=yp,
                                lhsT=xt[:, 2 * p + c, bs],
                                rhs=wcat_sb[:, p, c, :],
                                start=(i == 0),
                                stop=(i == 3 and not with_bias),
                                skip_group_check=True,
                            )
                    if with_bias:
                        nc.tensor.matmul(
                            out=yp,
                            lhsT=ones_sb,
                            rhs=beff_sb,
                            start=False, stop=True, skip_group_check=True,
                        )
                    # ---- LayerNorm stats: even cols = plane0, odd =
                    # plane1, so one single-group bn_stats gives both ----
                    if modes[sj] == "C":
                        yi = yip.tile([128, 2 * D], _BF16)
                        yis[sj] = yi
                        nc.scalar.copy(out=yi, in_=yp)
                    if "stats" not in skip:
                        bsrc = (yis[sj] if (modes[sj] == "C"
                                            and bn_src == "sbuf") else yp)
                        nc.vector.bn_stats(out=st[:, sj], in_=bsrc)

                # ---- stats tail: rstd = (cv/D+eps)^-0.5, nmr = -m*rstd ----
                # st[:, :, 1::3] = means, st[:, :, 2::3] = count*var
                if "stats" not in skip:
                    nc.scalar.activation(
                        out=rstd, in_=st[:, :, 2::3],
                        func=mybir.ActivationFunctionType.Sqrt,
                        bias=eps_sb, scale=1.0 / D,
                    )
                    nc.vector.reciprocal(out=rstd, in_=rstd)
                    nc.vector.scalar_tensor_tensor(
                        out=nmr, in0=st[:, :, 1::3], scalar=-1.0,
                        in1=rstd, op0=mul, op1=mul)

                # ---- normalize: (y*rstd + nmr), de-interleave PSUM fp32
                # -> SBUF bf16 ----
                for sj in range(SG):
                    j = g0 + sj
                    yp, ypv = yps[sj]
                    mode = modes[sj]
                    for h in range(2):
                        if "stats" in skip or "norm" in skip:
                            nc.scalar.copy(out=yo[:, j, h * D:(h + 1) * D],
                                           in_=ypv[:, h, :])
                        elif mode == "C":
                            # (y - mean) * rstd on the idle GPSIMD engine
                            # (nmr-free: Pool lacks scalar_tensor_tensor)
                            yiv = yis[sj].rearrange("p (f h) -> p h f", h=2)
                            nc.gpsimd.tensor_scalar(
                                out=yo[:, j, h * D:(h + 1) * D],
                                in0=yiv[:, h, :],
                                scalar1=st[:, sj, 1 + 3 * h:2 + 3 * h],
                                scalar2=rstd[:, sj, h:h + 1],
                                op0=mybir.AluOpType.subtract,
                                op1=mul,
                            )
                        elif (
                            (mode in ("D", "V") if h == 0
                             else mode in ("D", "S")) if mode
                            else (norm_split and h == 0
                                  and (m * G + j) % int(norm_split) == 0)
                        ):
                            nc.vector.tensor_scalar(
                                out=yo[:, j, h * D:(h + 1) * D],
                                in0=ypv[:, h, :],
                                scalar1=rstd[:, sj, h:h + 1],
                                scalar2=nmr[:, sj, h:h + 1],
                                op0=mul,
                                op1=mybir.AluOpType.add,
                            )
                        else:
                            nc.scalar.activation(
                                out=yo[:, j, h * D:(h + 1) * D],
                                in_=ypv[:, h, :],
                                func=mybir.ActivationFunctionType.Identity,
                                bias=nmr[:, sj, h:h + 1],
                                scale=rstd[:, sj, h:h + 1],
                            )
                    if with_gamma:
                        for h in range(2):
                            nc.vector.tensor_mul(
                                out=yo[:, j, h * D:(h + 1) * D],
                                in0=yo[:, j, h * D:(h + 1) * D],
                                in1=gamma_sb,
                            )
                    if with_beta:
                        for h in range(2):
                            nc.vector.tensor_add(
                                out=yo[:, j, h * D:(h + 1) * D],
                                in0=yo[:, j, h * D:(h + 1) * D],
                                in1=beta_sb,
                            )

            if repeats > 1:
                with tc.For_i(0, repeats, 1):
                    pending = []
                    for m in range(n_mega):
                        megatile(m)
                    for args in pending:
                        emit_store(*args)
            else:
                pending = []
                for m in range(n_mega):
                    megatile(m)
                for args in pending:
                    emit_store(*args)
    split_waits(nc)
    return nc


def _to_bf16(a):
    import ml_dtypes
    return a.astype(ml_dtypes.bfloat16)


def _prepare(inputs, G=2, split_mm=True, xt_pc=False, out_pc=False):
    """Host-side prep: collapse weights, bf16 conversion, pre-transpose,
    interleave weight columns, shard across cores."""
    x = np.asarray(inputs["x"], dtype=np.float32)
    ipw = np.asarray(inputs["in_proj_w"], dtype=np.float32)
    ipb = np.asarray(inputs["in_proj_b"], dtype=np.float32)
    opw = np.asarray(inputs["out_proj_w"], dtype=np.float32)
    opb = np.asarray(inputs["out_proj_b"], dtype=np.float32)
    gamma = np.asarray(inputs["ln_gamma"], dtype=np.float32)
    beta = np.asarray(inputs["ln_beta"], dtype=np.float32)

    d = x.shape[2]
    wv = ipw[2 * d:3 * d]
    bv = ipb[2 * d:3 * d]
    weff_t = np.ascontiguousarray((opw @ wv).T)          # [in_f, out_f]
    beff = opw @ bv + opb                                # [out_f]

    with_bias = bool(np.any(beff != 0.0))
    with_gamma = bool(np.any(gamma != 1.0))
    with_beta = bool(np.any(beta != 0.0))

    nb = x.shape[0]
    per_core = nb // N_CORES
    tile_rows = 128 * G

    # Pre-transposed bf16 x: [n_tiles, 2*d (f-major: plane*d+feat), rows]
    xt = _to_bf16(x).reshape(nb // tile_rows, tile_rows, 2 * d)
    xt = np.ascontiguousarray(xt.swapaxes(1, 2))
    if xt_pc:
        # partition-contiguous: [tiles, 128 part, 4 chunk, rows] so each
        # partition's DMA line is one contiguous run
        xt = np.ascontiguousarray(
            xt.reshape(nb // tile_rows, 4, 128, tile_rows).swapaxes(1, 2))

    if split_mm:
        base = {
            "wt": _to_bf16(weff_t.reshape(2, 128, d)),
            "ident": _to_bf16(np.eye(128, dtype=np.float32)),
        }
        if split_mm == "hybrid":
            eye = np.eye(d, dtype=np.float32)
            wc0 = np.empty((128, 2 * d), dtype=np.float32)
            wc0[:, 0::2] = eye[0:128]
            wc0[:, 1::2] = weff_t[0:128]
            base["wcat0"] = _to_bf16(wc0)
    else:
        # moving operands with plane-interleaved columns: for lhsT = x_p
        # chunk c, out col 2f+h gets (identity if h==p else Weff.T) col f
        eye = np.eye(d, dtype=np.float32)
        wcat = np.empty((2, 2, 128, 2 * d), dtype=np.float32)
        for c in range(2):
            rows = slice(c * 128, (c + 1) * 128)
            wcat[0, c, :, 0::2] = eye[rows]
            wcat[0, c, :, 1::2] = weff_t[rows]
            wcat[1, c, :, 0::2] = weff_t[rows]
            wcat[1, c, :, 1::2] = eye[rows]
        base = {"wcat": _to_bf16(wcat)}
    if with_bias:
        base["beff"] = _to_bf16(np.repeat(beff, 2).reshape(1, 2 * d))
    if with_gamma:
        base["gamma"] = np.ascontiguousarray(gamma.reshape(1, d))
    if with_beta:
        base["beta"] = np.ascontiguousarray(beta.reshape(1, d))

    tiles_per_core = per_core // tile_rows
    in_maps = []
    for c in range(N_CORES):
        m = dict(base)
        m["xt"] = xt[c * tiles_per_core:(c + 1) * tiles_per_core]
        in_maps.append(m)
    return in_maps, per_core, (with_bias, with_gamma, with_beta), x.shape


def kernel(x, in_proj_w, in_proj_b, out_proj_w, out_proj_b, ln_gamma, ln_beta,
           _trace=False, _G=2, _opts=None):
    opts = dict(split_mm=False)
    if _opts:
        opts.update(_opts)
    inputs = dict(x=x, in_proj_w=in_proj_w, in_proj_b=in_proj_b,
                  out_proj_w=out_proj_w, out_proj_b=out_proj_b,
                  ln_gamma=ln_gamma, ln_beta=ln_beta)
    in_maps, per_core, (wb, wg, wbt), xshape = _prepare(
        inputs, G=_G, split_mm=opts["split_mm"],
        xt_pc=opts.get("xt_pc", False), out_pc=opts.get("out_pc", False))
    nc = build_nc(per_core, G=_G, with_bias=wb, with_gamma=wg, with_beta=wbt,
                  **opts)
    res = bass_utils.run_bass_kernel_spmd(
        nc, in_maps, core_ids=list(range(N_CORES)), trace=_trace,
    )
    outs = [r["out"] for r in res.results]
    if opts.get("out_pc", False):
        # [n_mega, 128, G, 2D] -> [n, 2D]: row index is m*G*128 + g*128 + p
        outs = [o.transpose(0, 2, 1, 3).reshape(-1, o.shape[-1])
                for o in outs]
    out = np.concatenate(outs, axis=0)
    kernel.last_results = res
    return out.astype(np.float32).reshape(xshape)

